# revision 47
# baseline (speedup 1.0000x reference)
"""Trainium2 Bass kernel for MinibatchDiscrimination.

Reference computation:
    M = (x @ T).reshape(B, OUT_F, INTER_F)              # [128, 128, 32]
    l1[i,j,o] = sum_k |M[i,o,k] - M[j,o,k]|             # [128, 128, 128]
    o_b = sum_j exp(-l1) - 1                            # [128, 128]
    out = concat([x, o_b], axis=1)                      # [128, 1152]

Sharding: each of the 8 cores owns 16 of the 128 output features (o).

Circulant decomposition over the pair axis: with j = (i+d) mod B, the
symmetric pairwise matrix only needs offsets d = 1..64:
    D_d[i, (o,k)] = ((I - P_d)^T M)[i, (o,k)]       (P_d = rotate-by-d)
one PE matmul per d.  The abs+sum-over-k PSUM drain (the critical
resource: PSUM is readable only by DVE and ACT, one operand per
instruction, ~1 elem/cycle/lane each) is split across both:

  D-batches: DVE tensor_reduce(add, |.|) straight PSUM -> l1, fully
    fused, 1.04 ns/elem.
  A-batches (in adjacent pairs): ACT runs one Abs pass per batch
    (PSUM -> SBUF bf16, 0.83 ns/elem) into a shared pair tile, then
    DVE folds k with a 5-level bf16 tensor_tensor tree in 2x packed
    mode (~0.52 ns/elem) - about 2.9x cheaper per element for DVE
    than the fused reduce, so the two engines drain PSUM in parallel.

E = exp(-l1) on ACT, and the row sums
    o_b[i,o] = sum_{d=1}^{64} E_d[i,o] + sum_{d=1}^{63} E_d[(i-d)%B, o]
fold back on the PE as 64 accumulating matmuls with stationary
(I + P_d).  d never equals 0, so no self-similarity correction needed.

The +-1 permutation stationaries and T ship as fp8e4m3 (+-1 is exact;
T quantization perturbs l1 ~1%, far inside the exp underflow regime)
in single contiguous partition-major DMAs on one queue so they
complete in strict priority order.  l1 is kept in bf16 (values ~1e3,
absolute error ~8; exp(-l1) underflows to 0 either way).
The x-passthrough part of the output is done on host.
"""

import numpy as np

B = 128
IN_F = 1024
OUT_F = 128
INTER_F = 32
N_CORES = 8
O_PER_CORE = OUT_F // N_CORES  # 16 output features per core
COLS_PER_CORE = O_PER_CORE * INTER_F  # 512 columns of T per core
ND = B // 2  # 64 circulant offsets (d = 1..64)
DB = 4  # d's per PSUM batch
NB = ND // DB  # 16 batches
KK = IN_F // 128  # 8 contraction tiles
BCOLS = DB * O_PER_CORE  # 64 l1 columns per batch

# engine path per batch: D = fused DVE tensor_reduce; A = ACT Abs +
# DVE bf16 tree (A's grouped into adjacent runs sharing one abs tile).
# D's lead each cycle so DVE has work while ACT fills the first pair;
# the last two A's tree individually to shorten the DVE drain tail.
ENG = "ADAADAADAADAAAAA"
A_GROUPS = [(0,), (2, 3), (5, 6), (8, 9), (11, 12), (13, 14), (15,)]
assert len(ENG) == NB and ENG.count("A") == 12
assert sorted(b for g in A_GROUPS for b in g) == [
    i for i in range(NB) if ENG[i] == "A"
]

_cache = {}


def _build_bass():
    import concourse.bass as bass
    import concourse.bacc as bacc
    import concourse.tile as tile
    import concourse.mybir as mybir

    fp32 = mybir.dt.float32
    bf16 = mybir.dt.bfloat16
    fp8 = mybir.dt.float8e4

    nc = bacc.Bacc("TRN2")

    xe_in = nc.dram_tensor("xe", [128, KK * B], bf16, kind="ExternalInput")
    te_in = nc.dram_tensor("te", [128, KK * COLS_PER_CORE], fp8, kind="ExternalInput")
    difs_in = nc.dram_tensor("difs", [128, ND * B], fp8, kind="ExternalInput")
    sums_in = nc.dram_tensor("sums", [128, ND * B], fp8, kind="ExternalInput")
    ob_out = nc.dram_tensor("ob", [B, O_PER_CORE], fp32, kind="ExternalOutput")

    DIF0 = 16  # d's in the first (priority) difs chunk

    with tile.TileContext(nc) as tc:
        with (
            tc.tile_pool(name="const", bufs=1) as const_pool,
            tc.tile_pool(name="work", bufs=2) as work_pool,
            tc.tile_pool(name="psum", bufs=2, space="PSUM") as psum_pool,
        ):
            # ---- input DMAs: all on the sync queue -> strict priority order
            # (one HWDGE queue still fans out across all 16 DMA engines) ----
            xe_all = const_pool.tile([128, KK * B], bf16, tag="xe")
            te_all = const_pool.tile([128, KK * COLS_PER_CORE], fp8, tag="te")
            nc.sync.dma_start(xe_all[:], xe_in[:])
            nc.sync.dma_start(te_all[:], te_in[:])
            difs_all = const_pool.tile([128, ND * B], fp8, tag="difs")
            nc.sync.dma_start(difs_all[:, : DIF0 * B], difs_in[:, : DIF0 * B])
            nc.sync.dma_start(difs_all[:, DIF0 * B :], difs_in[:, DIF0 * B :])
            sums_all = const_pool.tile([128, ND * B], fp8, tag="sums")
            nc.sync.dma_start(sums_all[:], sums_in[:])

            # ---- PE warm-up: dummy matmuls on a never-written scratch tile
            # while the input DMAs run, so the p-state ramp (full clock after
            # ~3us of continuous execution) completes before stage 1 ----
            junk = const_pool.tile([128, COLS_PER_CORE], bf16, tag="junk")
            nc.gpsimd.memset(junk[:], 0.0)
            ps_m = psum_pool.tile([128, COLS_PER_CORE], fp32, tag="psd")
            for w in range(12):
                nc.tensor.matmul(
                    ps_m[:],
                    lhsT=junk[:, 0:B],
                    rhs=junk[:],
                    start=True,
                    stop=True,
                )

            # ---- stage 1: M = x @ T_c -> PSUM [128 (i), 512 (o,k)]; the
            # PSUM->SBUF cast is split across ACT and DVE halves ----
            HC = COLS_PER_CORE // 2
            m_sb = const_pool.tile([128, COLS_PER_CORE], bf16, tag="m_sb")
            for kk in range(KK):
                nc.tensor.matmul(
                    ps_m[:],
                    lhsT=xe_all[:, kk * B : (kk + 1) * B],
                    rhs=te_all[:, kk * COLS_PER_CORE : (kk + 1) * COLS_PER_CORE],
                    start=(kk == 0),
                    stop=(kk == KK - 1),
                )
            nc.scalar.copy(m_sb[:, :HC], ps_m[:, :HC])
            nc.vector.tensor_copy(m_sb[:, HC:], ps_m[:, HC:])

            # ---- d-loop over 16 batches of 4 d's ----
            l1_all = const_pool.tile([128, ND * O_PER_CORE], bf16, tag="l1")
            escr = const_pool.tile([128, ND * O_PER_CORE], bf16, tag="escr")
            EC = ND * O_PER_CORE // 4  # exp chunk: 4 batches
            grp_of = {}
            for g in A_GROUPS:
                for b in g:
                    grp_of[b] = g
            av = None
            for b in range(NB):
                ps = psum_pool.tile([128, DB * COLS_PER_CORE], fp32, tag="psd")
                for t in range(DB):
                    di = b * DB + t  # d = di + 1
                    nc.tensor.matmul(
                        ps[:, t * COLS_PER_CORE : (t + 1) * COLS_PER_CORE],
                        lhsT=difs_all[:, di * B : (di + 1) * B],
                        rhs=m_sb[:],
                        start=True, stop=True,
                    )
                if ENG[b] == "D":
                    with nc.allow_low_precision("l1 ~1e3; exp underflows either way"):
                        nc.vector.tensor_reduce(
                            l1_all[:, b * BCOLS : (b + 1) * BCOLS],
                            ps[:].rearrange(
                                "p (t o k) -> p t o k", o=O_PER_CORE, k=INTER_F
                            ),
                            axis=mybir.AxisListType.X,
                            op=mybir.AluOpType.add,
                            apply_absolute_value=True,
                        )
                else:
                    grp = grp_of[b]
                    QB = len(grp)
                    q = grp.index(b)
                    if q == 0:
                        av = work_pool.tile(
                            [128, QB * DB * COLS_PER_CORE], bf16, tag="av"
                        )
                    # two half-tile Abs ops: subtile deps let the first half
                    # start as soon as the batch's first two matmuls land
                    HB = DB * COLS_PER_CORE // 2
                    nc.scalar.activation(
                        av[:, q * DB * COLS_PER_CORE :][:, :HB],
                        ps[:, :HB],
                        mybir.ActivationFunctionType.Abs,
                    )
                    nc.scalar.activation(
                        av[:, q * DB * COLS_PER_CORE + HB :][:, :HB],
                        ps[:, HB:],
                        mybir.ActivationFunctionType.Abs,
                    )
                    if q == QB - 1:
                        # 5-level bf16 tree over k for the group;
                        # 2x packed mode on every level but the last
                        b0 = grp[0]
                        w3 = av[:].rearrange(
                            "p (d o k) -> p d o k", o=O_PER_CORE, k=INTER_F
                        )
                        NDQ = QB * DB * O_PER_CORE
                        t1 = work_pool.tile([128, NDQ * 16], bf16, tag="t1")
                        t13 = t1[:].rearrange(
                            "p (d o k) -> p d o k", o=O_PER_CORE, k=16
                        )
                        nc.vector.tensor_tensor(
                            t13, w3[:, :, :, 0:16], w3[:, :, :, 16:32],
                            mybir.AluOpType.add,
                        )
                        t2 = work_pool.tile([128, NDQ * 8], bf16, tag="t2")
                        t23 = t2[:].rearrange(
                            "p (d o k) -> p d o k", o=O_PER_CORE, k=8
                        )
                        nc.vector.tensor_tensor(
                            t23, t13[:, :, :, 0:8], t13[:, :, :, 8:16],
                            mybir.AluOpType.add,
                        )
                        t3 = work_pool.tile([128, NDQ * 4], bf16, tag="t3")
                        t33 = t3[:].rearrange(
                            "p (d o k) -> p d o k", o=O_PER_CORE, k=4
                        )
                        nc.vector.tensor_tensor(
                            t33, t23[:, :, :, 0:4], t23[:, :, :, 4:8],
                            mybir.AluOpType.add,
                        )
                        t4 = work_pool.tile([128, NDQ * 2], bf16, tag="t4")
                        t43 = t4[:].rearrange(
                            "p (d o k) -> p d o k", o=O_PER_CORE, k=2
                        )
                        nc.vector.tensor_tensor(
                            t43, t33[:, :, :, 0:2], t33[:, :, :, 2:4],
                            mybir.AluOpType.add,
                        )
                        l1g = l1_all[:, b0 * BCOLS :][:, : QB * BCOLS].rearrange(
                            "p (d o k) -> p d o k", o=O_PER_CORE, k=1
                        )
                        nc.vector.tensor_tensor(
                            l1g, t43[:, :, :, 0:1], t43[:, :, :, 1:2],
                            mybir.AluOpType.add,
                        )
                # exp for chunk g, one chunk late so the ACT stream never
                # blocks upcoming Abs work
                if b % 4 == 3 and b >= 7:
                    g = b // 4 - 1
                    nc.scalar.activation(
                        escr[:, g * EC : (g + 1) * EC],
                        l1_all[:, g * EC : (g + 1) * EC],
                        mybir.ActivationFunctionType.Exp,
                        scale=-1.0,
                    )
            # final exp split: batches 12-14 can run while DVE still trees
            # batch 15, leaving only 64 columns on the critical tail
            nc.scalar.activation(
                escr[:, 3 * EC : 15 * BCOLS],
                l1_all[:, 3 * EC : 15 * BCOLS],
                mybir.ActivationFunctionType.Exp,
                scale=-1.0,
            )
            nc.scalar.activation(
                escr[:, 15 * BCOLS :],
                l1_all[:, 15 * BCOLS :],
                mybir.ActivationFunctionType.Exp,
                scale=-1.0,
            )

            # ---- o_b = sum_d (I + P_d)^T E_d, accumulated on the PE ----
            ps_ob = psum_pool.tile([128, O_PER_CORE], fp32, tag="psd")
            for di in range(ND):
                nc.tensor.matmul(
                    ps_ob[:],
                    lhsT=sums_all[:, di * B : (di + 1) * B],
                    rhs=escr[:, di * O_PER_CORE : (di + 1) * O_PER_CORE],
                    start=(di == 0),
                    stop=(di == ND - 1),
                )
            obf = const_pool.tile([128, O_PER_CORE], fp32, tag="obf")
            nc.vector.tensor_copy(obf[:], ps_ob[:])
            nc.sync.dma_start(ob_out[:], obf[:])

    nc.finalize()
    return nc


def _prep_inputs(x, T):
    import ml_dtypes

    bf16 = ml_dtypes.bfloat16
    fp8 = ml_dtypes.float8_e4m3fn

    # xe[c, kk*B + i] = x[i, kk*128 + c]
    xe = np.ascontiguousarray(
        x.reshape(B, KK, 128).transpose(2, 1, 0).reshape(128, KK * B)
    ).astype(bf16)

    # difs[c, (d-1)*B + i] = delta(c==i) - delta(c==(i+d)%B)
    # sums[r, (d-1)*B + i] = delta(r==i) + (d<64)*delta(r==(i-d)%B)
    i_idx = np.arange(B)
    difs = np.zeros((B, ND * B), dtype=np.float32)
    sums = np.zeros((B, ND * B), dtype=np.float32)
    for d in range(1, ND + 1):
        col = (d - 1) * B + i_idx
        difs[i_idx, col] += 1.0
        difs[(i_idx + d) % B, col] -= 1.0
        sums[i_idx, col] += 1.0
        if d < ND:
            sums[(i_idx - d) % B, col] += 1.0
    difs = difs.astype(fp8)
    sums = sums.astype(fp8)

    in_maps = []
    for c in range(N_CORES):
        # te[cc, kk*512 + col] = T[kk*128 + cc, core_cols[col]]
        tc_block = T[:, c * COLS_PER_CORE : (c + 1) * COLS_PER_CORE]
        te = np.ascontiguousarray(
            tc_block.reshape(KK, 128, COLS_PER_CORE)
            .transpose(1, 0, 2)
            .reshape(128, KK * COLS_PER_CORE)
        ).astype(fp8)
        in_maps.append({"xe": xe, "te": te, "difs": difs, "sums": sums})
    return in_maps


def _install_ntff_hook_shim():
    """Register the axon NTFF profile hook (test-only; used when trace=True).

    The boot package ships the ctypes hook but the image's antenv lacks the
    axon_hooks module concourse imports it from; provide it via sys.modules.
    """
    import sys
    import types

    if "antenv.axon_hooks" in sys.modules:
        return
    try:
        sys.path.insert(0, "/root/.axon_site")
        from trn_agent_boot.trn_boot import _ntff_profile_via_ctypes

        so_path = "/opt/axon/libaxon_pjrt.so"
        hook = _ntff_profile_via_ctypes(so_path)
        mod = types.ModuleType("antenv.axon_hooks")
        mod.get_axon_ntff_profile_hook = lambda: hook
        mod.set_axon_ntff_profile_hook = lambda h: None
        sys.modules["antenv.axon_hooks"] = mod
    except Exception as e:  # profiling is best-effort
        print(f"ntff hook shim failed: {e}")


def _run(x, T, trace=False):
    from concourse.bass_utils import run_bass_kernel_spmd

    if trace:
        _install_ntff_hook_shim()
    if "nc" not in _cache:
        _cache["nc"] = _build_bass()
    nc = _cache["nc"]
    in_maps = _prep_inputs(x, T)
    res = run_bass_kernel_spmd(nc, in_maps, list(range(N_CORES)), trace=trace)
    ob = np.concatenate([res.results[c]["ob"] for c in range(N_CORES)], axis=1)
    out = np.concatenate([x.astype(np.float32), ob.astype(np.float32)], axis=1)
    return out, res


def kernel(x, T):
    x = np.asarray(x, dtype=np.float32)
    T = np.asarray(T, dtype=np.float32)
    out, _ = _run(x, T, trace=False)
    return out


# revision 48
# speedup vs baseline: 1.0757x; 1.0757x over previous
"""Trainium2 Bass kernel for MinibatchDiscrimination.

Reference computation:
    M = (x @ T).reshape(B, OUT_F, INTER_F)              # [128, 128, 32]
    l1[i,j,o] = sum_k |M[i,o,k] - M[j,o,k]|             # [128, 128, 128]
    o_b = sum_j exp(-l1) - 1                            # [128, 128]
    out = concat([x, o_b], axis=1)                      # [128, 1152]

Sharding: each of the 8 cores owns 16 of the 128 output features (o).

Circulant decomposition over the pair axis: with j = (i+d) mod B, the
symmetric pairwise matrix only needs offsets d = 1..64:
    D_d[i, (o,k)] = ((I - P_d)^T M)[i, (o,k)]       (P_d = rotate-by-d)
one PE matmul per d.  The abs+sum-over-k PSUM drain (the critical
resource: PSUM is readable only by DVE and ACT, one operand per
instruction, ~1 elem/cycle/lane each) is split across both:

  D-batches: DVE tensor_reduce(add, |.|) straight PSUM -> l1, fully
    fused, 1.04 ns/elem.
  A-batches (in adjacent pairs): ACT runs one Abs pass per batch
    (PSUM -> SBUF bf16, 0.83 ns/elem) into a shared pair tile, then
    DVE folds k with a 5-level bf16 tensor_tensor tree in 2x packed
    mode (~0.52 ns/elem) - about 2.9x cheaper per element for DVE
    than the fused reduce, so the two engines drain PSUM in parallel.

E = exp(-l1) on ACT, and the row sums
    o_b[i,o] = sum_{d=1}^{64} E_d[i,o] + sum_{d=1}^{63} E_d[(i-d)%B, o]
fold back on the PE as 64 accumulating matmuls with stationary
(I + P_d).  d never equals 0, so no self-similarity correction needed.

The +-1 permutation stationaries and T ship as fp8e4m3 (+-1 is exact;
T quantization perturbs l1 ~1%, far inside the exp underflow regime)
in single contiguous partition-major DMAs on one queue so they
complete in strict priority order.  l1 is kept in bf16 (values ~1e3,
absolute error ~8; exp(-l1) underflows to 0 either way).
The x-passthrough part of the output is done on host.
"""

import numpy as np

B = 128
IN_F = 1024
OUT_F = 128
INTER_F = 32
N_CORES = 8
O_PER_CORE = OUT_F // N_CORES  # 16 output features per core
COLS_PER_CORE = O_PER_CORE * INTER_F  # 512 columns of T per core
ND = B // 2  # 64 circulant offsets (d = 1..64)
DB = 4  # d's per PSUM batch
NB = ND // DB  # 16 batches
KK = IN_F // 128  # 8 contraction tiles
BCOLS = DB * O_PER_CORE  # 64 l1 columns per batch

# engine path per batch: D = fused DVE tensor_reduce; A = ACT Abs +
# DVE bf16 tree (A's grouped into adjacent runs sharing one abs tile).
# D's lead each cycle so DVE has work while ACT fills the first pair;
# the last two A's tree individually to shorten the DVE drain tail.
ENG = "ADAADAADAADAAAAA"
A_GROUPS = [(0,), (2, 3), (5, 6), (8, 9), (11, 12), (13, 14), (15,)]
assert len(ENG) == NB and ENG.count("A") == 12
assert sorted(b for g in A_GROUPS for b in g) == [
    i for i in range(NB) if ENG[i] == "A"
]

_cache = {}


def _build_bass():
    import concourse.bass as bass
    import concourse.bacc as bacc
    import concourse.tile as tile
    import concourse.mybir as mybir

    fp32 = mybir.dt.float32
    bf16 = mybir.dt.bfloat16
    fp8 = mybir.dt.float8e4

    nc = bacc.Bacc("TRN2")

    xe_in = nc.dram_tensor("xe", [128, KK * B], bf16, kind="ExternalInput")
    te_in = nc.dram_tensor("te", [128, KK * COLS_PER_CORE], fp8, kind="ExternalInput")
    difs_in = nc.dram_tensor("difs", [128, ND * B], fp8, kind="ExternalInput")
    sums_in = nc.dram_tensor("sums", [128, ND * B], fp8, kind="ExternalInput")
    ob_out = nc.dram_tensor("ob", [B, O_PER_CORE], fp32, kind="ExternalOutput")

    DIF0 = 16  # d's in the first (priority) difs chunk

    with tile.TileContext(nc) as tc:
        with (
            tc.tile_pool(name="const", bufs=1) as const_pool,
            tc.tile_pool(name="work", bufs=2) as work_pool,
            tc.tile_pool(name="psum", bufs=2, space="PSUM") as psum_pool,
        ):
            # ---- input DMAs: all on the sync queue -> strict priority order
            # (one HWDGE queue still fans out across all 16 DMA engines) ----
            xe_all = const_pool.tile([128, KK * B], bf16, tag="xe")
            te_all = const_pool.tile([128, KK * COLS_PER_CORE], fp8, tag="te")
            nc.sync.dma_start(xe_all[:], xe_in[:])
            nc.sync.dma_start(te_all[:], te_in[:])
            difs_all = const_pool.tile([128, ND * B], fp8, tag="difs")
            nc.sync.dma_start(difs_all[:, : DIF0 * B], difs_in[:, : DIF0 * B])
            nc.sync.dma_start(difs_all[:, DIF0 * B :], difs_in[:, DIF0 * B :])
            sums_all = const_pool.tile([128, ND * B], fp8, tag="sums")
            nc.sync.dma_start(sums_all[:], sums_in[:])

            # ---- PE warm-up: dummy matmuls on a never-written scratch tile
            # while the input DMAs run, so the p-state ramp (full clock after
            # ~3us of continuous execution) completes before stage 1 ----
            junk = const_pool.tile([128, COLS_PER_CORE], bf16, tag="junk")
            nc.gpsimd.memset(junk[:], 0.0)
            ps_m = psum_pool.tile([128, COLS_PER_CORE], fp32, tag="psd")
            for w in range(12):
                nc.tensor.matmul(
                    ps_m[:],
                    lhsT=junk[:, 0:B],
                    rhs=junk[:],
                    start=True,
                    stop=True,
                )

            # ---- stage 1: M = x @ T_c -> PSUM [128 (i), 512 (o,k)]; the
            # PSUM->SBUF cast is split across ACT and DVE halves ----
            HC = COLS_PER_CORE // 2
            m_sb = const_pool.tile([128, COLS_PER_CORE], bf16, tag="m_sb")
            for kk in range(KK):
                nc.tensor.matmul(
                    ps_m[:],
                    lhsT=xe_all[:, kk * B : (kk + 1) * B],
                    rhs=te_all[:, kk * COLS_PER_CORE : (kk + 1) * COLS_PER_CORE],
                    start=(kk == 0),
                    stop=(kk == KK - 1),
                )
            nc.scalar.copy(m_sb[:, :HC], ps_m[:, :HC])
            nc.vector.tensor_copy(m_sb[:, HC:], ps_m[:, HC:])

            # ---- d-loop over 16 batches of 4 d's ----
            l1_all = const_pool.tile([128, ND * O_PER_CORE], bf16, tag="l1")
            escr = const_pool.tile([128, ND * O_PER_CORE], bf16, tag="escr")
            EC = ND * O_PER_CORE // 4  # exp chunk: 4 batches
            grp_of = {}
            for g in A_GROUPS:
                for b in g:
                    grp_of[b] = g
            av = None
            for b in range(NB):
                ps = psum_pool.tile([128, DB * COLS_PER_CORE], fp32, tag="psd")
                for t in range(DB):
                    di = b * DB + t  # d = di + 1
                    nc.tensor.matmul(
                        ps[:, t * COLS_PER_CORE : (t + 1) * COLS_PER_CORE],
                        lhsT=difs_all[:, di * B : (di + 1) * B],
                        rhs=m_sb[:],
                        start=True, stop=True,
                    )
                if ENG[b] == "D":
                    with nc.allow_low_precision("l1 ~1e3; exp underflows either way"):
                        nc.vector.tensor_reduce(
                            l1_all[:, b * BCOLS : (b + 1) * BCOLS],
                            ps[:].rearrange(
                                "p (t o k) -> p t o k", o=O_PER_CORE, k=INTER_F
                            ),
                            axis=mybir.AxisListType.X,
                            op=mybir.AluOpType.add,
                            apply_absolute_value=True,
                        )
                else:
                    grp = grp_of[b]
                    QB = len(grp)
                    q = grp.index(b)
                    if q == 0:
                        av = work_pool.tile(
                            [128, QB * DB * COLS_PER_CORE], bf16, tag="av"
                        )
                    nc.scalar.activation(
                        av[:, q * DB * COLS_PER_CORE : (q + 1) * DB * COLS_PER_CORE],
                        ps[:],
                        mybir.ActivationFunctionType.Abs,
                    )
                    if q == QB - 1:
                        # 5-level bf16 tree over k for the group;
                        # 2x packed mode on every level but the last
                        b0 = grp[0]
                        w3 = av[:].rearrange(
                            "p (d o k) -> p d o k", o=O_PER_CORE, k=INTER_F
                        )
                        NDQ = QB * DB * O_PER_CORE
                        t1 = work_pool.tile([128, NDQ * 16], bf16, tag="t1")
                        t13 = t1[:].rearrange(
                            "p (d o k) -> p d o k", o=O_PER_CORE, k=16
                        )
                        nc.vector.tensor_tensor(
                            t13, w3[:, :, :, 0:16], w3[:, :, :, 16:32],
                            mybir.AluOpType.add,
                        )
                        t2 = work_pool.tile([128, NDQ * 8], bf16, tag="t2")
                        t23 = t2[:].rearrange(
                            "p (d o k) -> p d o k", o=O_PER_CORE, k=8
                        )
                        nc.vector.tensor_tensor(
                            t23, t13[:, :, :, 0:8], t13[:, :, :, 8:16],
                            mybir.AluOpType.add,
                        )
                        t3 = work_pool.tile([128, NDQ * 4], bf16, tag="t3")
                        t33 = t3[:].rearrange(
                            "p (d o k) -> p d o k", o=O_PER_CORE, k=4
                        )
                        nc.vector.tensor_tensor(
                            t33, t23[:, :, :, 0:4], t23[:, :, :, 4:8],
                            mybir.AluOpType.add,
                        )
                        t4 = work_pool.tile([128, NDQ * 2], bf16, tag="t4")
                        t43 = t4[:].rearrange(
                            "p (d o k) -> p d o k", o=O_PER_CORE, k=2
                        )
                        nc.vector.tensor_tensor(
                            t43, t33[:, :, :, 0:2], t33[:, :, :, 2:4],
                            mybir.AluOpType.add,
                        )
                        l1g = l1_all[:, b0 * BCOLS :][:, : QB * BCOLS].rearrange(
                            "p (d o k) -> p d o k", o=O_PER_CORE, k=1
                        )
                        nc.vector.tensor_tensor(
                            l1g, t43[:, :, :, 0:1], t43[:, :, :, 1:2],
                            mybir.AluOpType.add,
                        )
                # exp for chunk g, one chunk late so the ACT stream never
                # blocks upcoming Abs work
                if b % 4 == 3 and b >= 7:
                    g = b // 4 - 1
                    nc.scalar.activation(
                        escr[:, g * EC : (g + 1) * EC],
                        l1_all[:, g * EC : (g + 1) * EC],
                        mybir.ActivationFunctionType.Exp,
                        scale=-1.0,
                    )
            # final exp split: batches 12-14 can run while DVE still trees
            # batch 15, leaving only 64 columns on the critical tail
            nc.scalar.activation(
                escr[:, 3 * EC : 15 * BCOLS],
                l1_all[:, 3 * EC : 15 * BCOLS],
                mybir.ActivationFunctionType.Exp,
                scale=-1.0,
            )
            nc.scalar.activation(
                escr[:, 15 * BCOLS :],
                l1_all[:, 15 * BCOLS :],
                mybir.ActivationFunctionType.Exp,
                scale=-1.0,
            )

            # ---- o_b = sum_d (I + P_d)^T E_d, accumulated on the PE ----
            ps_ob = psum_pool.tile([128, O_PER_CORE], fp32, tag="psd")
            for di in range(ND):
                nc.tensor.matmul(
                    ps_ob[:],
                    lhsT=sums_all[:, di * B : (di + 1) * B],
                    rhs=escr[:, di * O_PER_CORE : (di + 1) * O_PER_CORE],
                    start=(di == 0),
                    stop=(di == ND - 1),
                )
            obf = const_pool.tile([128, O_PER_CORE], fp32, tag="obf")
            nc.vector.tensor_copy(obf[:], ps_ob[:])
            nc.sync.dma_start(ob_out[:], obf[:])

    nc.finalize()
    return nc


def _prep_inputs(x, T):
    import ml_dtypes

    bf16 = ml_dtypes.bfloat16
    fp8 = ml_dtypes.float8_e4m3fn

    # xe[c, kk*B + i] = x[i, kk*128 + c]
    xe = np.ascontiguousarray(
        x.reshape(B, KK, 128).transpose(2, 1, 0).reshape(128, KK * B)
    ).astype(bf16)

    # difs[c, (d-1)*B + i] = delta(c==i) - delta(c==(i+d)%B)
    # sums[r, (d-1)*B + i] = delta(r==i) + (d<64)*delta(r==(i-d)%B)
    i_idx = np.arange(B)
    difs = np.zeros((B, ND * B), dtype=np.float32)
    sums = np.zeros((B, ND * B), dtype=np.float32)
    for d in range(1, ND + 1):
        col = (d - 1) * B + i_idx
        difs[i_idx, col] += 1.0
        difs[(i_idx + d) % B, col] -= 1.0
        sums[i_idx, col] += 1.0
        if d < ND:
            sums[(i_idx - d) % B, col] += 1.0
    difs = difs.astype(fp8)
    sums = sums.astype(fp8)

    in_maps = []
    for c in range(N_CORES):
        # te[cc, kk*512 + col] = T[kk*128 + cc, core_cols[col]]
        tc_block = T[:, c * COLS_PER_CORE : (c + 1) * COLS_PER_CORE]
        te = np.ascontiguousarray(
            tc_block.reshape(KK, 128, COLS_PER_CORE)
            .transpose(1, 0, 2)
            .reshape(128, KK * COLS_PER_CORE)
        ).astype(fp8)
        in_maps.append({"xe": xe, "te": te, "difs": difs, "sums": sums})
    return in_maps


def _install_ntff_hook_shim():
    """Register the axon NTFF profile hook (test-only; used when trace=True).

    The boot package ships the ctypes hook but the image's antenv lacks the
    axon_hooks module concourse imports it from; provide it via sys.modules.
    """
    import sys
    import types

    if "antenv.axon_hooks" in sys.modules:
        return
    try:
        sys.path.insert(0, "/root/.axon_site")
        from trn_agent_boot.trn_boot import _ntff_profile_via_ctypes

        so_path = "/opt/axon/libaxon_pjrt.so"
        hook = _ntff_profile_via_ctypes(so_path)
        mod = types.ModuleType("antenv.axon_hooks")
        mod.get_axon_ntff_profile_hook = lambda: hook
        mod.set_axon_ntff_profile_hook = lambda h: None
        sys.modules["antenv.axon_hooks"] = mod
    except Exception as e:  # profiling is best-effort
        print(f"ntff hook shim failed: {e}")


def _run(x, T, trace=False):
    from concourse.bass_utils import run_bass_kernel_spmd

    if trace:
        _install_ntff_hook_shim()
    if "nc" not in _cache:
        _cache["nc"] = _build_bass()
    nc = _cache["nc"]
    in_maps = _prep_inputs(x, T)
    res = run_bass_kernel_spmd(nc, in_maps, list(range(N_CORES)), trace=trace)
    ob = np.concatenate([res.results[c]["ob"] for c in range(N_CORES)], axis=1)
    out = np.concatenate([x.astype(np.float32), ob.astype(np.float32)], axis=1)
    return out, res


def kernel(x, T):
    x = np.asarray(x, dtype=np.float32)
    T = np.asarray(T, dtype=np.float32)
    out, _ = _run(x, T, trace=False)
    return out
